# revision 37
# baseline (speedup 1.0000x reference)
"""Trainium2 Bass kernel for nn_MixedAttention_16561393893612.

Computation (reference semantics, fp32 inputs):
  x [B=4, T=2048, D=1024]; first n_s=1984 tokens share QKV weights W_s
  [3D, D]; the last 64 tokens each have their own W_ns[t] [3D, D]; full
  softmax attention (H=16 heads, Dh=64) over all T; out-proj W_out [D, D].

Sharding: tensor-parallel over heads. Core c owns heads (2c, 2c+1):
  - QKV projection for its 128-row m-slice of W_s / W_ns (all tokens)
  - full attention for its 2 heads
  - out-proj partial with the matching 128-column slice of W_out;
    host sums the 8 partial outputs.

Layout strategy (all heavy matmuls contract over n=D on partitions):
  - host pre-transposes x (xT), the W_s/W_ns/W_out slices, so no
    on-chip transposes are needed;
  - Q^T/K^T are produced [m(128=2 heads), t] so scores come out as
    S^T [k, q] with softmax along free axis impossible -- instead we
    exp() without max-subtraction (inputs are unit-scale; scores*0.125
    are bounded ~|4|) and get the softmax denominator from an appended
    ones-column in V via the same PV matmul (row 64 of psO = sum_k P).
  - PV uses V in token-major layout [k, dh+1] as lhsT giving O^T [d, q],
    which feeds the out-proj directly (contraction over d).

Compute dtype bf16 (PE streams 1 elem/cycle; fp32 would be 4 passes),
fp32 PSUM accumulation everywhere, fp32 output partials.
"""

import os
import numpy as np
import ml_dtypes

import bass_rust
import concourse.bass as bass
import concourse.mybir as mybir
import concourse.tile as tile
from concourse.bass_utils import run_bass_kernel_spmd

FP32 = mybir.dt.float32
BF16 = mybir.dt.bfloat16
NPBF16 = ml_dtypes.bfloat16

B, T, D = 4, 2048, 1024
H, DH = 16, 64
NS = 64
N_S = T - NS  # 1984
NCORES = 8
HPC = H // NCORES     # heads per core = 2
M = HPC * DH          # 128: per-core m-slice width per q/k/v
NT = D // 128         # 8 n-tiles (contraction)
KT = T // 128         # 16 k-tiles
QTW = 512             # q tile width
QT = T // QTW         # 4 q tiles
# q-chunks: the first four cover [0, n_s) and run k-tiles 0..14 during the
# projection phase (no ns tokens touched); the 64-wide tail plus k-tile 15
# run after the ns patch.
QCH = [(0, 512), (512, 512), (1024, 512), (1536, 448), (N_S, 64)]
SCALE = 1.0 / np.sqrt(DH).astype(np.float32)

# The walrus build on this image rejects instructions carrying more than
# one sync wait ("Too many sync wait commands").  Tile freely emits
# multi-wait instructions, so after tracing we hoist excess waits onto
# injected same-engine NoOps placed immediately before the instruction
# (each engine executes its block-order subsequence, so the waits still
# complete before the instruction issues).
_MAX_WAITS = 1


def _split_waits(nc, max_waits=_MAX_WAITS):
    ctr = 0
    for f in nc.m.functions:
        for blk in f.blocks:
            newlist = []
            for inst in blk.instructions:
                si = inst.sync_info
                waits = list(si.on_wait) if si else []
                if len(waits) > max_waits:
                    head, keep = waits[:-max_waits], waits[-max_waits:]
                    for i in range(0, len(head), max_waits):
                        chunk = head[i : i + max_waits]
                        nop = mybir.InstNoOp(name=f"W-split-{ctr}", ins=[], outs=[])
                        ctr += 1
                        nop.engine = inst.engine
                        nop.sync_info = mybir.SyncInfo(on_wait=chunk, on_update=[])
                        newlist.append(nop)
                    inst.sync_info = mybir.SyncInfo(
                        on_wait=keep, on_update=list(si.on_update)
                    )
                newlist.append(inst)
            blk.instructions[:] = newlist
    return ctr


def _build_program():
    nc = bass.Bass()
    xT_d = nc.dram_tensor("xT", [B, QT, NT, 128, QTW], BF16, kind="ExternalInput")
    wq_d = nc.dram_tensor("wq", [128, NT, M], BF16, kind="ExternalInput")
    wk_d = nc.dram_tensor("wk", [128, NT, M], BF16, kind="ExternalInput")
    wv_d = nc.dram_tensor("wv", [128, NT, M], BF16, kind="ExternalInput")
    wo_d = nc.dram_tensor("wo", [M, D], BF16, kind="ExternalInput")
    wns_d = nc.dram_tensor("wns", [NS, NT, 128, 3 * M], BF16, kind="ExternalInput")
    xns_d = nc.dram_tensor("xns", [128, NS, NT, B], BF16, kind="ExternalInput")
    y_d = nc.dram_tensor("y", [B, T, D], BF16, kind="ExternalOutput")
    ns_scr = nc.dram_tensor("ns_scratch", [B, NS, 3 * M], BF16)

    from contextlib import ExitStack

    with tile.TileContext(nc) as tc, ExitStack() as ctx:
        sing = ctx.enter_context(tc.tile_pool(name="sing", bufs=1))
        xpool = ctx.enter_context(tc.tile_pool(name="xpool", bufs=24))
        wnspool = ctx.enter_context(tc.tile_pool(name="wnspool", bufs=6))
        ptpool = ctx.enter_context(tc.tile_pool(name="ptpool", bufs=4))
        evac = ctx.enter_context(tc.tile_pool(name="evac", bufs=3))
        otp = ctx.enter_context(tc.tile_pool(name="otp", bufs=4))
        small = ctx.enter_context(tc.tile_pool(name="small", bufs=4))
        lpool = ctx.enter_context(tc.tile_pool(name="lpool", bufs=2))
        otup = ctx.enter_context(tc.tile_pool(name="otup", bufs=2))
        ps_big = ctx.enter_context(tc.tile_pool(name="ps_big", bufs=2, space="PSUM"))
        ps_pj = ctx.enter_context(tc.tile_pool(name="ps_pj", bufs=2, space="PSUM"))
        ps_o = ctx.enter_context(tc.tile_pool(name="ps_o", bufs=2, space="PSUM"))

        # ---- constants / persistent tensors ----
        wq_sb = sing.tile([128, NT, M], BF16)
        wk_sb = sing.tile([128, NT, M], BF16)
        wv_sb = sing.tile([128, NT, M], BF16)
        wo_sb = sing.tile([M, D], BF16)
        nc.sync.dma_start(wq_sb, wq_d[:])
        nc.sync.dma_start(wk_sb, wk_d[:])
        nc.sync.dma_start(wv_sb, wv_d[:])
        nc.sync.dma_start(wo_sb, wo_d[:])

        QT_sb = sing.tile([M, B, T], BF16)          # [m(q rows), b, t]
        KT_sb = sing.tile([M, B, T], BF16)          # [m(k rows), b, t]
        V_sb = sing.tile([128, B, KT, HPC, DH + 1], BF16)  # token-major V
        nc.gpsimd.memset(V_sb[:, :, :, :, DH : DH + 1], 1.0)

        xns_sb = sing.tile([128, NS, NT, B], BF16)
        nc.sync.dma_start(xns_sb, xns_d[:])
        from concourse.masks import make_identity

        ident = sing.tile([128, 128], BF16)
        make_identity(nc, ident)
        ones_sb = sing.tile([1, DH], BF16)
        nc.gpsimd.memset(ones_sb, 1.0)

        # SBUF-resident pre-attn partial accumulators (bf16): one per
        # (b, q-chunk<4, h); merged into the post-patch psO with a DVE add
        # instead of a DRAM round-trip.
        spill = {}
        for b_ in range(B):
            for qc_ in range(4):
                for h_ in range(HPC):
                    spill[(b_, qc_, h_)] = sing.tile(
                        [DH + 1, QCH[qc_][1]], BF16, name=f"spl_{b_}_{qc_}_{h_}"
                    )

        # ---- ns projection: out[b, m] per ns-token, lhsT = x cols ----
        # results staged to DRAM so they can be re-loaded transposed.
        # Emitted in groups interleaved with the per-batch projections so
        # the 50MB wns stream overlaps projection compute instead of
        # blocking the PE queue at the head of the kernel.
        def ns_group(tp_lo, tp_hi):
            for tp in range(tp_lo, tp_hi):
                wt = wnspool.tile(
                    [128, NT, 3 * M], BF16, tag="wns", name=f"wns_{tp}"
                )
                # one DMA per nt chunk: a single 0.8MB transfer is stuck on
                # one of the 16 DMA queues (~22GB/s each); eight contiguous
                # 98KB transfers land across queues ~7x sooner
                for nt in range(NT):
                    nc.sync.dma_start(wt[:, nt, :], wns_d[tp, nt])
                psn = ps_pj.tile([B, 3 * M], FP32, tag="pj", name=f"psn_{tp}")
                for nt in range(NT):
                    nc.tensor.matmul(
                        psn,
                        lhsT=xns_sb[:, tp, nt, :],
                        rhs=wt[:, nt, :],
                        start=(nt == 0),
                        stop=(nt == NT - 1),
                    )
                nst = small.tile([B, 3 * M], BF16, tag="nst", name=f"nst_{tp}")
                nc.vector.tensor_copy(nst, psn)
                nc.sync.dma_start(ns_scr[:, tp, :], nst)

        # one attention step for a q-window: S matmuls for a group of
        # k-tiles into one psum, one batched exp, chained PV accumulations
        def attn_step(b, psO, q0, qw, kts, kt_first, kt_last, name):
            # per-kt slice width: matmul outputs must not straddle a PSUM
            # bank boundary mid-slice, so pad non-bank-sized q-widths to 512
            sw = qw if (qw % 512 == 0 or len(kts) * qw <= 512) else 512
            psS = [
                ps_big.tile(
                    [128, len(kts) * sw], FP32, tag="mm", name=f"psS_{name}_{h}"
                )
                for h in range(HPC)
            ]
            for j, kt in enumerate(kts):
                for h in range(HPC):
                    nc.tensor.matmul(
                        psS[h][:, j * sw : j * sw + qw],
                        lhsT=KT_sb[
                            h * DH : (h + 1) * DH, b, kt * 128 : (kt + 1) * 128
                        ],
                        rhs=QT_sb[h * DH : (h + 1) * DH, b, q0 : q0 + qw],
                        start=True,
                        stop=True,
                    )
            for h in range(HPC):
                pt = ptpool.tile([128, len(kts) * sw], BF16, tag="pt")
                # exp over the whole (possibly padded) tile; pad columns
                # hold stale values whose exp is never read
                nc.scalar.activation(
                    pt, psS[h], mybir.ActivationFunctionType.Exp, scale=float(SCALE)
                )
                for j, kt in enumerate(kts):
                    nc.tensor.matmul(
                        psO[h],
                        lhsT=V_sb[:, b, kt, h, :],
                        rhs=pt[:, j * sw : j * sw + qw],
                        start=(kt == kt_first),
                        stop=(kt == kt_last),
                    )

        # evacuate psO (optionally merging a pre-attn spill), compute the
        # softmax normalizer 1/l with DVE transposes, and write OT columns
        def finish_chunk(b, OT, c0, psO, qw, spill_key, name):
            for h in range(HPC):
                otu = otup.tile([DH + 32, QTW], FP32, tag=f"otu{h}")
                if spill_key is not None:
                    nc.vector.tensor_add(
                        otu[0 : DH + 1, 0:qw], psO[h], spill[spill_key + (h,)]
                    )
                else:
                    nc.vector.tensor_copy(otu[0 : DH + 1, 0:qw], psO[h])
                # 1/l without DRAM bounces: a DVE 32x32 block-transpose
                # moves the l row into columns (l[32j+a] -> ltr[a, 32j]),
                # reciprocal runs on those columns, a second transpose
                # lands 1/l as a contiguous row on partition 0, and a
                # rank-1 matmul broadcasts it over the 64 head dims.
                ltr = lpool.tile([32, QTW], FP32, tag="ltr")
                nc.vector.transpose(ltr[:, 0:qw], otu[DH : DH + 32, 0:qw])
                linv = lpool.tile([32, QTW], BF16, tag="linv")
                with nc.allow_low_precision(reason="1/l broadcast in bf16"):
                    nc.vector.reciprocal(
                        linv[:, 0:qw].rearrange("p (j c) -> p j c", c=32)[:, :, 0:1],
                        ltr[:, 0:qw].rearrange("p (j c) -> p j c", c=32)[:, :, 0:1],
                    )
                lrow = lpool.tile([32, QTW], BF16, tag="lrow")
                nc.vector.transpose(lrow[:, 0:qw], linv[:, 0:qw])
                # broadcast row 0 over the 64 head dims with two DVE
                # stream-shuffles (within-32-block permutation per quadrant)
                recb = lpool.tile([DH, QTW], BF16, tag="recb")
                nc.vector.stream_shuffle(recb[0:32, 0:qw], lrow[:, 0:qw], [0] * 32)
                nc.vector.stream_shuffle(
                    recb[32:DH, 0:qw], recb[0:32, 0:qw], [0] * 32
                )
                nc.vector.tensor_mul(
                    OT[h * DH : (h + 1) * DH, c0 : c0 + qw],
                    otu[0:DH, 0:qw],
                    recb[:, 0:qw],
                )

        def oproj_chunk(b, OT, i, tch):
            yt = evac.tile([128, D], BF16, tag="y")
            for e in range(D // QTW):
                psY = ps_pj.tile([128, QTW], FP32, tag="pj")
                nc.tensor.matmul(
                    psY,
                    lhsT=OT[:, i * 128 : (i + 1) * 128],
                    rhs=wo_sb[:, e * QTW : (e + 1) * QTW],
                    start=True,
                    stop=True,
                )
                # split evacuation across DVE and ACT
                if e % 2 == 0:
                    nc.vector.tensor_copy(yt[:, e * QTW : (e + 1) * QTW], psY)
                else:
                    nc.scalar.activation(
                        yt[:, e * QTW : (e + 1) * QTW],
                        psY,
                        mybir.ActivationFunctionType.Copy,
                    )
            nc.gpsimd.dma_start(y_d[b, tch * 128 : (tch + 1) * 128, :], yt)

        # pre-patch k-tile groups: pairs (0,1)..(12,13) then 14 solo
        PRE_KTS = [(2 * i, 2 * i + 1) for i in range(7)] + [(14,)]

        GRP = NS // B
        # a couple of ns tokens up front give the PE work while the first
        # x chunks stream in
        ns_group(0, 2)
        for b in range(B):
            # ---- shared QKV projection for batch b ----
            proj_scope = nc.named_scope(f"proj_b{b}")
            proj_scope.__enter__()
            xts = [[None] * NT for _ in range(QT)]
            for qt in range(QT):
                for nt in range(NT):
                    xt = xpool.tile([128, QTW], BF16, tag="xt")
                    nc.sync.dma_start(xt, xT_d[b, qt, nt])
                    xts[qt][nt] = xt
            for qt in range(QT):
                for w_sb, out_sb in ((wq_sb, QT_sb), (wk_sb, KT_sb)):
                    ps = ps_pj.tile([M, QTW], FP32, tag="pj")
                    for nt in range(NT):
                        nc.tensor.matmul(
                            ps,
                            lhsT=w_sb[:, nt, :],
                            rhs=xts[qt][nt],
                            start=(nt == 0),
                            stop=(nt == NT - 1),
                        )
                    nc.vector.tensor_copy(
                        out_sb[:, b, qt * QTW : (qt + 1) * QTW], ps
                    )
                for i in range(QTW // 128):
                    tch = qt * (QTW // 128) + i
                    ps = ps_pj.tile([128, M], FP32, tag="pj")
                    for nt in range(NT):
                        nc.tensor.matmul(
                            ps,
                            lhsT=xts[qt][nt][:, i * 128 : (i + 1) * 128],
                            rhs=wv_sb[:, nt, :],
                            start=(nt == 0),
                            stop=(nt == NT - 1),
                        )
                    nc.vector.tensor_copy(
                        V_sb[:, b, tch, :, 0:DH],
                        ps.rearrange("p (h d) -> p h d", h=HPC),
                    )

            # pre-patch attention: q in [0, n_s) x k-tiles 0..14 touch no
            # ns tokens, so they run during the DMA-bound head phase;
            # partial [65, qw] accumulators spill to SBUF in bf16
            for qc in range(4):
                q0, qw = QCH[qc]
                psO = [
                    ps_o.tile([DH + 1, qw], FP32, tag="psO", name=f"psOp_{b}_{qc}_{h}")
                    for h in range(HPC)
                ]
                for kts in PRE_KTS:
                    attn_step(b, psO, q0, qw, kts, 0, 14, f"p{b}_{qc}_{kts[0]}")
                for h in range(HPC):
                    nc.vector.tensor_copy(spill[(b, qc, h)], psO[h])
            # interleave a quarter of the ns-token projections per batch
            # (keeps the wns DMA stream flowing under projection compute)
            ns_group(2 if b == 0 else b * GRP, (b + 1) * GRP)
            proj_scope.__exit__(None, None, None)

        # ---- patch ns tokens (last 64) from the staged ns results ----
        # Q/K need a [t', m] -> [m, t'] transpose: PE-transpose beats a
        # 2-byte-granularity DMA gather by ~40x here
        for b in range(B):
            for j, out_sb in ((0, QT_sb), (1, KT_sb)):
                nsp = small.tile([NS, M], BF16, tag="nsp", name=f"nsp_{b}_{j}")
                nc.sync.dma_start(nsp, ns_scr[b, :, j * M : (j + 1) * M])
                pst = ps_pj.tile([M, NS], BF16, tag="pj", name=f"pst_{b}_{j}")
                nc.tensor.transpose(pst, nsp, ident[0:NS, 0:NS])
                nc.vector.tensor_copy(out_sb[:, b, N_S:T], pst)
            for h in range(HPC):
                nc.sync.dma_start(
                    V_sb[DH : 2 * DH, b, KT - 1, h, 0:DH],
                    ns_scr[b, :, 2 * M + h * DH : 2 * M + (h + 1) * DH],
                )

        # ---- attention tail: k-tile 15 for the pre-patched q-chunks plus
        # the full pass for the 64 ns q-rows.  Units are round-robined
        # across batches with the out-proj deferred one unit so the
        # in-order PE queue never head-of-line blocks on a finish chain.
        tail_scope = nc.named_scope("tail")
        tail_scope.__enter__()
        # per-(b, qc) OT tiles; both heads stacked on partitions so the
        # out-proj contracts over the full 128 rows in one matmul.  The
        # qc3 tile also receives the ns q-rows at columns 448:512.
        OTs = {}
        pending = []

        def flush_pending():
            for b_, OTt_, i_, tch_ in pending:
                oproj_chunk(b_, OTt_, i_, tch_)
            pending.clear()

        for qc in range(4):
            q0, qw = QCH[qc]
            for b in range(B):
                OTt = otp.tile([128, QTW], BF16, tag="ot", name=f"ot_{b}_{qc}")
                OTs[(b, qc)] = OTt
                psO = [
                    ps_o.tile([DH + 1, qw], FP32, tag="psO", name=f"psO_{b}_{qc}_{h}")
                    for h in range(HPC)
                ]
                attn_step(b, psO, q0, qw, (15,), 15, 15, f"t{b}_{qc}")
                finish_chunk(b, OTt, 0, psO, qw, (b, qc), f"t{b}_{qc}")
                flush_pending()
                n_tch = qw // 128 if qc < 3 else 3
                pending.extend(
                    (b, OTt, i, q0 // 128 + i) for i in range(n_tch)
                )
        # ns q-rows: full 16 k-tiles in two 8-tile steps, written into the
        # qc3 tile at columns 448:512 so token chunk 15 stays contiguous
        q0, qw = QCH[4]
        for b in range(B):
            OTt = OTs[(b, 3)]
            psO = [
                ps_o.tile([DH + 1, qw], FP32, tag="psO", name=f"psO_{b}_ns_{h}")
                for h in range(HPC)
            ]
            attn_step(b, psO, q0, qw, tuple(range(8)), 0, 15, f"n{b}_0")
            attn_step(b, psO, q0, qw, tuple(range(8, 16)), 0, 15, f"n{b}_1")
            finish_chunk(b, OTt, 448, psO, qw, None, f"n{b}")
            flush_pending()
            pending.append((b, OTt, 3, KT - 1))
        flush_pending()
        tail_scope.__exit__(None, None, None)

    _split_waits(nc)
    return nc


_NC_CACHE = None
LAST_RESULTS = None


def _prep_inputs(x, W_s, W_ns, W_out):
    """Slice/transpose/cast the full inputs into per-core input maps."""
    x = np.asarray(x, dtype=np.float32)
    W_s = np.asarray(W_s, dtype=np.float32)
    W_ns = np.asarray(W_ns, dtype=np.float32)
    W_out = np.asarray(W_out, dtype=np.float32)

    xb = x.astype(NPBF16)
    # xT[b, qt, nt, p, q] = x[b, qt*512+q, nt*128+p]
    xT = np.ascontiguousarray(
        xb.transpose(0, 2, 1)
        .reshape(B, NT, 128, QT, QTW)
        .transpose(0, 3, 1, 2, 4)
    )
    # xns[p, t', nt, b] = x[b, n_s+t', nt*128+p]
    xns = np.ascontiguousarray(
        xb[:, N_S:, :].transpose(2, 1, 0).reshape(NT, 128, NS, B).transpose(1, 2, 0, 3)
    )
    wnsb = W_ns.astype(NPBF16)
    wsb = W_s.astype(NPBF16)
    wob = W_out.astype(NPBF16)

    in_maps = []
    for c in range(NCORES):
        r0 = c * M
        sel = np.r_[r0 : r0 + M, D + r0 : D + r0 + M, 2 * D + r0 : 2 * D + r0 + M]

        def wslice(rows):
            # [128 rows m, 1024 n] -> [128 p(n), NT, m]
            w = wsb[rows, :]  # [M, D]
            return np.ascontiguousarray(
                w.T.reshape(NT, 128, M).transpose(1, 0, 2)
            )

        wq = wslice(slice(r0, r0 + M))
        wk = wslice(slice(D + r0, D + r0 + M))
        wv = wslice(slice(2 * D + r0, 2 * D + r0 + M))
        # wo[r, e] = W_out[e, c*128 + r]  (r = h*64 + d, matching OT rows)
        wo = np.ascontiguousarray(wob[:, c * M : (c + 1) * M].T)
        # wns[t', nt, p, m] = W_ns[t', sel[m], n=nt*128+p]
        wns = np.ascontiguousarray(
            wnsb[:, sel, :].transpose(0, 2, 1).reshape(NS, NT, 128, 3 * M)
        )
        in_maps.append(
            {"xT": xT, "wq": wq, "wk": wk, "wv": wv, "wo": wo, "wns": wns, "xns": xns}
        )
    return in_maps


def kernel(x, n_s, W_s, W_ns, W_out):
    global _NC_CACHE, LAST_RESULTS
    assert int(n_s) == N_S, f"kernel compiled for n_s={N_S}, got {int(n_s)}"
    in_maps = _prep_inputs(x, W_s, W_ns, W_out)
    if _NC_CACHE is None:
        _NC_CACHE = _build_program()
    nc = _NC_CACHE
    trace = os.environ.get("BASS_TRACE", "") not in ("", "0")
    kwargs = {}
    if trace:
        stitch = os.environ.get("BASS_STITCH", "") not in ("", "0")
        kwargs = dict(
            trace=True, trace_cores=list(range(NCORES)), stitch_traces=stitch
        )
    res = run_bass_kernel_spmd(nc, in_maps, core_ids=list(range(NCORES)), **kwargs)
    LAST_RESULTS = res
    out = np.zeros((B, T, D), dtype=np.float32)
    for c in range(NCORES):
        out += res.results[c]["y"].astype(np.float32)
    return out

